# revision 17
# baseline (speedup 1.0000x reference)
"""CGConv-style GNN encoder (2 x (CGConv + BatchNorm) + global mean pool) on
8 TRN2 NeuronCores.

Sharding: nodes are padded and split into 8 contiguous per-core shards; each
edge is owned by the core that owns its dst node, so the scatter-add is core
local.  Small weights are replicated; only BatchNorm statistics are
all-reduced (layer 1; layer 2's BN is folded into the pooled output on the
host, which is exact because pooling is linear).  The updated node features
are all-gathered in bf16 between the layers.

Per 128-edge tile on device:
  - x[src]/x[dst] rows are fetched with dma_gather(transpose=True) from bf16
    tables in internal DRAM, landing channel-major [128c, 128e].  Tables are
    kept below 8MB inside 16MB-aligned windows (the gather ucode corrupts
    addresses whose byte offset has bits 23 and 15 both set).
  - Three matmuls (dst rows, src rows, edge_attr+bias row) accumulate
    z @ [Wf | Ws] into a PSUM slab [128e, 256].
  - sigma(f) = 1 / (1 + exp(-f)) via ACT Exp + DVE reciprocal_approx_fast;
    softplus(s) = Ln(Exp(s) + 1) - both live in the natural_log_exp ACT
    table, so there are no activation-table switches.
  - The scatter-add is a matmul with a one-hot matrix U[e, n] built by a
    single DVE tensor_scalar is_equal against an iota row, accumulated into
    a per-block PSUM tile.
"""

import numpy as np
import ml_dtypes

import concourse.bacc as bacc
import concourse.mybir as mybir
import concourse.tile as tile
from concourse.bass_utils import run_bass_kernel_spmd
from contextlib import ExitStack

F32 = mybir.dt.float32
BF16 = mybir.dt.bfloat16
I16 = mybir.dt.int16
AF = mybir.ActivationFunctionType
ALU = mybir.AluOpType
BF16NP = ml_dtypes.bfloat16

NCORES = 8
D = 128


def _setup_act_tables():
    """Reorder act_info.json so Exp/Ln/Copy all resolve to one table set
    (natural_log_exp_and_others), eliminating per-op ACT_TABLE_LOADs."""
    import json, shutil
    import concourse.hw_specs as hw_specs
    import concourse.bacc as _bacc
    from neuronxcc.driver.Job import Job
    from neuronxcc.driver.jobs.support.FindActInfo import findActInfoFile

    if os.environ.get("BASS_ACT_ROOT_JSON_PATH"):
        return
    src = findActInfoFile(Job.getPackageDir(), "gen3")
    dstdir = "/tmp/act_custom"
    os.makedirs(dstdir, exist_ok=True)
    d = json.load(open(src))
    order = sorted(range(len(d["act_func_sets"])),
                   key=lambda i: d["act_func_sets"][i]["name"] != "natural_log_exp_and_others")
    d["act_func_sets"] = [d["act_func_sets"][i] for i in order]
    with open(os.path.join(dstdir, "act_info.json"), "w") as f:
        json.dump(d, f)
    srcdir = os.path.dirname(src)
    for fn in os.listdir(srcdir):
        if fn != "act_info.json":
            tgt = os.path.join(dstdir, fn)
            if not os.path.exists(tgt):
                os.symlink(os.path.join(srcdir, fn), tgt)
    os.environ["BASS_ACT_ROOT_JSON_PATH"] = os.path.join(dstdir, "act_info.json")

    import concourse.mybir as _mybir
    def _tables(arch, _d=d):
        return {
            ent["name"]: {
                _mybir.ActivationFunctionType.from_pwp(v) for v in ent["act"].keys()
            }
            for ent in _d["act_func_sets"]
        }
    hw_specs.get_activation_tables = _tables
    _bacc.get_activation_tables = _tables


import os
_setup_act_tables()
DE = 64
EPS = 1e-5
SLAB = 4  # tiles per activation slab


def _ceil(a, b):
    return -(-a // b)


# ---------------------------------------------------------------------------
# host-side data prep
# ---------------------------------------------------------------------------

def _prep(x, edge_index, edge_attr, batch, G):
    N = x.shape[0]
    NBC = _ceil(_ceil(N, 128), NCORES)
    NPC = NBC * 128
    NP = NPC * NCORES
    HALF = NP // 2

    src = np.asarray(edge_index[0], np.int64)
    dst = np.asarray(edge_index[1], np.int64)
    ea = np.asarray(edge_attr, np.float32)
    batch = np.asarray(batch, np.int64)

    core_of = dst // NPC
    dst_loc = dst - core_of * NPC
    blk = dst_loc >> 7
    half = (src >= HALF).astype(np.int64)

    keys = core_of * (NBC * 2) + blk * 2 + half
    order = np.lexsort((src, keys))
    src_s, dstl_s = src[order], dst_loc[order]
    ea_s = ea[order]
    keys_s = keys[order]

    counts = np.zeros((NCORES, NBC, 2), np.int64)
    np.add.at(counts, (core_of, blk, half), 1)
    m = _ceil(counts.max(axis=0), 128)  # [NBC, 2] tiles per group
    T = int(m.sum())
    goff = np.zeros((NBC, 2), np.int64)  # group start, in edges
    acc = 0
    for b in range(NBC):
        for h in (0, 1):
            goff[b, h] = acc
            acc += m[b, h] * 128

    idx_src = np.zeros((NCORES, T * 128), np.int64)
    dstcol = np.full((NCORES, T * 128), 1024.0, np.float32)
    ea_flat = np.zeros((NCORES, T * 128 * 66), BF16NP)

    bounds = np.searchsorted(keys_s, np.arange(NCORES * NBC * 2 + 1))
    for c in range(NCORES):
        for b in range(NBC):
            for h in (0, 1):
                k = c * (NBC * 2) + b * 2 + h
                lo, hi = bounds[k], bounds[k + 1]
                n = hi - lo
                o = int(goff[b, h])
                L = int(m[b, h]) * 128
                idx_src[c, o:o + n] = src_s[lo:hi] - h * HALF
                dstcol[c, o:o + n] = (dstl_s[lo:hi] - b * 128).astype(np.float32)
                # group ea block [66, L] at flat offset 66*o:
                # row 64 = 1.0 (bias), row 65 = dst-in-block (1024 for pads)
                blkea = np.zeros((66, L), np.float32)
                blkea[:64, :n] = ea_s[lo:hi].T
                blkea[64, :] = 1.0
                blkea[65, :] = 1024.0
                blkea[65, :n] = (dstl_s[lo:hi] - b * 128).astype(np.float32)
                ea_flat[c, 66 * o: 66 * (o + L)] = blkea.astype(BF16NP).ravel()

    def wrap16(v):  # [T*128] -> [128, T*8]
        return np.tile(v.reshape(-1, 16).T.astype(np.int16), (8, 1))

    prep = dict(N=N, NBC=NBC, NPC=NPC, NP=NP, HALF=HALF, m=m, T=T, goff=goff)
    prep["idx_src_w"] = np.stack([wrap16(idx_src[c]) for c in range(NCORES)])
    prep["idx_seq_w"] = wrap16(np.arange(NPC, dtype=np.int64))
    prep["dstcol"] = np.ascontiguousarray(
        dstcol.reshape(NCORES, T, 128).transpose(0, 2, 1))
    prep["ea_flat"] = ea_flat

    xpad = np.zeros((NP, D), np.float32)
    xpad[:N] = np.asarray(x, np.float32)
    prep["x_shard"] = xpad.reshape(NCORES, NPC, D).copy()

    ids = np.arange(NP).reshape(NCORES, NBC, 128)
    prep["mask"] = np.ascontiguousarray(
        (ids < N).astype(np.float32).transpose(0, 2, 1))

    gnode = np.full(NP, -1, np.int64)
    gnode[:N] = batch
    glo = np.zeros(NCORES, np.int64)
    GW = 1
    for c in range(NCORES):
        gs = gnode[c * NPC:(c + 1) * NPC]
        gs = gs[gs >= 0]
        if len(gs):
            glo[c] = int(gs.min())
            GW = max(GW, int(gs.max() - gs.min() + 1))
    Bmat = np.zeros((NCORES, NBC * 128, GW), np.float32)
    for c in range(NCORES):
        gs = gnode[c * NPC:(c + 1) * NPC]
        rows = np.nonzero(gs >= 0)[0]
        Bmat[c, rows, gs[rows] - glo[c]] = 1.0
    prep["Bmat"], prep["GW"], prep["glo"] = Bmat, GW, glo
    prep["cnts"] = np.bincount(batch, minlength=G).astype(np.float32)
    return prep


def _wcat(Wf, bf, Ws, bs):
    Wf = np.asarray(Wf, np.float32)
    Ws = np.asarray(Ws, np.float32)
    wd = np.concatenate([Wf[0:D], Ws[0:D]], axis=1)
    ws = np.concatenate([Wf[D:2 * D], Ws[D:2 * D]], axis=1)
    we = np.zeros((65, 2 * D), np.float32)
    we[:64] = np.concatenate([Wf[2 * D:], Ws[2 * D:]], axis=1)
    we[64, :D] = np.asarray(bf, np.float32)
    we[64, D:] = np.asarray(bs, np.float32)
    return (wd.astype(BF16NP), ws.astype(BF16NP), we.astype(BF16NP))


# ---------------------------------------------------------------------------
# device program
# ---------------------------------------------------------------------------

def _build(prep, debug=False):
    N = prep["N"]
    NBC, NPC, NP, HALF = prep["NBC"], prep["NPC"], prep["NP"], prep["HALF"]
    m, T, goff, GW = prep["m"], prep["T"], prep["goff"], prep["GW"]
    Lmax = int(m.max()) * 128

    nc = bacc.Bacc("TRN2", target_bir_lowering=False, debug=False,
                   num_devices=NCORES, num_swdge_queues=4)

    # ---- I/O
    x_in = nc.dram_tensor("x_shard", [NPC, D], F32, kind="ExternalInput")
    isrc = nc.dram_tensor("idx_src", [128, T * 8], I16, kind="ExternalInput")
    iseq = nc.dram_tensor("idx_seq", [128, NPC // 16], I16, kind="ExternalInput")
    dcol = nc.dram_tensor("dstcol", [128, T], F32, kind="ExternalInput")
    eain = nc.dram_tensor("ea_flat", [T * 128 * 66], BF16, kind="ExternalInput")
    mask_in = nc.dram_tensor("mask", [128, NBC], F32, kind="ExternalInput")
    bmat_in = nc.dram_tensor("Bmat", [NBC * 128, GW], F32, kind="ExternalInput")
    w_ins = []
    for l in range(2):
        w_ins.append((
            nc.dram_tensor(f"wd{l}", [128, 256], BF16, kind="ExternalInput"),
            nc.dram_tensor(f"ws{l}", [128, 256], BF16, kind="ExternalInput"),
            nc.dram_tensor(f"we{l}", [65, 256], BF16, kind="ExternalInput"),
        ))
    bng_in = nc.dram_tensor("bn_g", [1, D], F32, kind="ExternalInput")
    bnb_in = nc.dram_tensor("bn_b", [1, D], F32, kind="ExternalInput")
    iota_in = nc.dram_tensor("iota", [128, 128], BF16, kind="ExternalInput")
    iotaT_in = nc.dram_tensor("iotaT", [128, 128], BF16, kind="ExternalInput")

    pool_out = nc.dram_tensor("pool_out", [GW, D], F32, kind="ExternalOutput")
    stats2_out = nc.dram_tensor("stats2", [1, 256], F32, kind="ExternalOutput")
    if debug:
        dbg_y1 = nc.dram_tensor("dbg_y1", [NPC, D], F32, kind="ExternalOutput")
        dbg_gs = nc.dram_tensor("dbg_gs", [128, 256], F32, kind="ExternalOutput")
        dbg_gd = nc.dram_tensor("dbg_gd", [128, 256], F32, kind="ExternalOutput")
        dbg_msg = nc.dram_tensor("dbg_msg", [128, 256], F32, kind="ExternalOutput")
        dbg_agg = nc.dram_tensor("dbg_agg", [128, D], F32, kind="ExternalOutput")
        dbg_ar = nc.dram_tensor("dbg_ar", [1, 256], F32, kind="ExternalOutput")
        dbg_st = nc.dram_tensor("dbg_st", [1, 256], F32, kind="ExternalOutput")

    # ---- internal DRAM: gather tables below 8MB of 16MB-aligned windows
    WIN = 16 * 1024 * 1024
    SAFE = 8 * 1024 * 1024

    def win_tables(specs):
        out = []
        pos = nc.local_dram_base
        pad = (-pos) % WIN
        if pad:
            nc.dram_tensor(f"_pad{len(out)}_{specs[0][0]}", [pad], mybir.dt.uint8)
        used = 0
        for name, rows in specs:
            nbytes = rows * D * 2
            assert used + nbytes <= SAFE, (name, rows)
            out.append(nc.dram_tensor(name, [rows, D], BF16))
            used += nbytes
            used += (-used) % 4096
        return out

    (xA,) = win_tables([("xA", HALF)])
    (xB,) = win_tables([("xB", HALF)])
    (yA,) = win_tables([("yA", HALF)])
    (yB,) = win_tables([("yB", HALF)])
    x_own, y_own = win_tables([("x_own", NPC), ("y_own", NPC)])

    y1_res = nc.dram_tensor("y1_res", [NPC, D], F32)
    rowscr = nc.dram_tensor("rowscr", [2, 128], F32)
    ag_out = nc.dram_tensor("ag_out", [NP, D], BF16, addr_space="Shared")
    ar_in = nc.dram_tensor("ar_in", [1, 256], F32)
    ar_out = nc.dram_tensor("ar_out", [1, 256], F32, addr_space="Shared")
    rg = [list(range(NCORES))]

    with tile.TileContext(nc) as tc, ExitStack() as ctx:
        const = ctx.enter_context(tc.tile_pool(name="const", bufs=1))
        gat = ctx.enter_context(tc.tile_pool(name="gat", bufs=4))
        work = ctx.enter_context(tc.tile_pool(name="work", bufs=3))
        blkp = ctx.enter_context(tc.tile_pool(name="blkp", bufs=3))
        ps_slab = ctx.enter_context(tc.tile_pool(name="ps_slab", bufs=2, space="PSUM"))
        ps_agg = ctx.enter_context(tc.tile_pool(name="ps_agg", bufs=1, space="PSUM"))
        ps_misc = ctx.enter_context(tc.tile_pool(name="ps_misc", bufs=1, space="PSUM"))
        ps_pool = ctx.enter_context(tc.tile_pool(name="ps_pool", bufs=1, space="PSUM"))

        # ---- constants and resident streams
        W = []
        for l in range(2):
            wd = const.tile([128, 256], BF16, tag=f"wd{l}")
            ws = const.tile([128, 256], BF16, tag=f"ws{l}")
            we = const.tile([65, 256], BF16, tag=f"we{l}")
            nc.sync.dma_start(out=wd[:], in_=w_ins[l][0][:])
            nc.sync.dma_start(out=ws[:], in_=w_ins[l][1][:])
            nc.sync.dma_start(out=we[:], in_=w_ins[l][2][:])
            W.append((wd, ws, we))
        iota_t = const.tile([128, 128], BF16, tag="iota")
        nc.sync.dma_start(out=iota_t[:], in_=iota_in[:])
        ones_c = const.tile([128, 1], F32, tag="ones_c")
        nc.gpsimd.memset(ones_c[:], 1.0)
        ones_r = const.tile([1, 128], F32, tag="ones_r")
        nc.gpsimd.memset(ones_r[:], 1.0)
        negone_c = const.tile([128, 1], F32, tag="negone_c")
        nc.gpsimd.memset(negone_c[:], -1.0)
        eps_r = const.tile([1, 1], F32, tag="eps_r")
        nc.gpsimd.memset(eps_r[:], EPS)
        neghalf_r = const.tile([1, 1], F32, tag="neghalf_r")
        nc.gpsimd.memset(neghalf_r[:], -0.5)
        negone_b = const.tile([128, 1], BF16, tag="negone_b")
        nc.gpsimd.memset(negone_b[:], -1.0)
        bng_t = const.tile([1, D], F32, tag="bng")
        nc.sync.dma_start(out=bng_t[:], in_=bng_in[:])
        bnb_t = const.tile([1, D], F32, tag="bnb")
        nc.sync.dma_start(out=bnb_t[:], in_=bnb_in[:])
        mask_t = const.tile([128, NBC], F32, tag="mask")
        nc.sync.dma_start(out=mask_t[:], in_=mask_in[:])
        is_t = const.tile([128, T * 8], I16, tag="is")
        nc.sync.dma_start(out=is_t[:], in_=isrc[:])
        iq_t = const.tile([128, NPC // 16], I16, tag="iq")
        nc.sync.dma_start(out=iq_t[:], in_=iseq[:])
        iotaT_t = const.tile([128, 128], BF16, tag="iotaT")
        nc.sync.dma_start(out=iotaT_t[:], in_=iotaT_in[:])
        dc_t = const.tile([128, T], F32, tag="dc")
        nc.sync.dma_start(out=dc_t[:], in_=dcol[:])
        S1_t = const.tile([128, 128], F32, tag="S1")
        B1_t = const.tile([128, 128], F32, tag="B1")

        # ---- prologue: own x shard -> bf16 table; AllGather; split to xA/xB
        for b in range(NBC):
            xt = blkp.tile([128, D], F32, tag="xc")
            nc.sync.dma_start(out=xt[:], in_=x_in[b * 128:(b + 1) * 128, :])
            xb = blkp.tile([128, D], BF16, tag="xcb")
            nc.vector.tensor_copy(xb[:], xt[:])
            nc.sync.dma_start(out=x_own[b * 128:(b + 1) * 128, :], in_=xb[:])
        nc.gpsimd.collective_compute(
            "AllGather", ALU.bypass, replica_groups=rg,
            ins=[x_own[:, :]], outs=[ag_out[:, :]])
        nc.sync.dma_start(out=xA[:, :], in_=ag_out[0:HALF, :])
        nc.sync.dma_start(out=xB[:, :], in_=ag_out[HALF:NP, :])

        # ---------------- layer body ----------------
        self_qn = [0]
        GQ = int(os.environ.get("GQ", "1"))

        def layer(l, tblA, tblB, tbl_own, stats_ps):
            wd, ws, we = W[l]
            # own-shard transposed features, resident for the layer
            xoT = const.tile([128, NPC], BF16, tag="xoT")
            for q0 in range(0, NPC, 512):
                q1 = min(q0 + 512, NPC)
                nc.gpsimd.dma_gather(
                    out_ap=xoT[:, q0:q1].rearrange("p (o k) -> p o k", o=1),
                    in_ap=tbl_own[:, :],
                    idxs_ap=iq_t[:, q0 // 16:q1 // 16],
                    num_idxs=q1 - q0, num_idxs_reg=q1 - q0,
                    elem_size=D, transpose=True, queue_num=0)
            for b in range(NBC):
                agg = ps_agg.tile([128, 128], F32, tag="agg")
                # per-block dst-projection table P = x_block @ [Wf|Ws]_dst
                pps = ps_misc.tile([128, 256], F32, tag="pps")
                nc.tensor.matmul(pps[:], lhsT=xoT[:, b * 128:(b + 1) * 128],
                                 rhs=wd[:], start=True, stop=True)
                P_sb = blkp.tile([128, 256], BF16, tag="P_sb")
                nc.vector.tensor_copy(P_sb[:], pps[:])
                tiles = []  # (tile_global, gather_tile, col_in_gather)
                for h in (0, 1):
                    mb = int(m[b, h])
                    if mb == 0:
                        continue
                    L = mb * 128
                    o = int(goff[b, h])
                    gtbl = tblA if h == 0 else tblB
                    GCH = 512  # dma_gather hangs the device at num_idxs >= 768
                    gs = gat.tile([128, Lmax], BF16, tag="gs")
                    for q0 in range(0, L, GCH):
                        q1 = min(q0 + GCH, L)
                        nc.gpsimd.dma_gather(
                            out_ap=gs[:, q0:q1].rearrange("p (o k) -> p o k", o=1),
                            in_ap=gtbl[:, :],
                            idxs_ap=is_t[:, (o + q0) // 16:(o + q1) // 16],
                            num_idxs=q1 - q0, num_idxs_reg=q1 - q0,
                            elem_size=D, transpose=True,
                            queue_num=self_qn[0] % GQ)
                        self_qn[0] += 1
                    eat = gat.tile([66, Lmax], BF16, tag="eat")
                    nc.sync.dma_start(
                        out=eat[:, :L],
                        in_=eain[66 * o:66 * (o + L)].rearrange("(p e) -> p e", p=66))
                    rep = gat.tile([128, Lmax], BF16, tag="rep")
                    nc.sync.dma_start(
                        out=rep[:, :L],
                        in_=eain[66 * o + 65 * L:66 * o + 66 * L]
                        .rearrange("(o e) -> o e", o=1).to_broadcast([128, L]))
                    if debug and l == 0 and b == 0 and h == 0:
                        dgs = blkp.tile([128, 256], F32, tag="dgs")
                        nc.vector.tensor_copy(dgs[:], gs[:, :256])
                        nc.sync.dma_start(out=dbg_gs[:], in_=dgs[:])
                        dgd = blkp.tile([128, 256], F32, tag="dgd")
                        nc.vector.tensor_copy(dgd[:], rep[:, :256])
                        nc.sync.dma_start(out=dbg_gd[:], in_=dgd[:])
                    for j in range(mb):
                        tiles.append((o // 128 + j, (gs, rep, eat), j))
                nt = len(tiles)
                HS = SLAB * 128
                sig_insts, ln_insts = [], []
                for s0 in range(0, nt, SLAB):
                    ns = min(SLAB, nt - s0)
                    nf = ns * 128
                    slab = ps_slab.tile([128, 2 * HS], F32, tag="slab")
                    UTs = []
                    for jj in range(ns):
                        tg, (gs, rep, eat), j = tiles[s0 + jj]
                        fsl = slice(jj * 128, jj * 128 + 128)
                        ssl = slice(HS + jj * 128, HS + jj * 128 + 128)
                        esl = slice(j * 128, (j + 1) * 128)
                        UT = work.tile([128, 128], BF16, tag="UT")
                        nc.vector.tensor_tensor(
                            out=UT[:], in0=iotaT_t[:], in1=rep[:, esl],
                            op=ALU.is_equal)
                        UTs.append(UT)
                        nc.tensor.matmul(slab[:, fsl], lhsT=UT[:], rhs=P_sb[:, 0:128],
                                         start=True, stop=False)
                        nc.tensor.matmul(slab[:, fsl], lhsT=gs[:, esl], rhs=ws[:, 0:128],
                                         start=False, stop=False)
                        nc.tensor.matmul(slab[:, fsl], lhsT=eat[:65, esl], rhs=we[:, 0:128],
                                         start=False, stop=True)
                        nc.tensor.matmul(slab[:, ssl], lhsT=UT[:], rhs=P_sb[:, 128:256],
                                         start=True, stop=False)
                        nc.tensor.matmul(slab[:, ssl], lhsT=gs[:, esl], rhs=ws[:, 128:256],
                                         start=False, stop=False)
                        nc.tensor.matmul(slab[:, ssl], lhsT=eat[:65, esl], rhs=we[:, 128:256],
                                         start=False, stop=True)
                    sf = work.tile([128, HS], BF16, tag="sf")
                    i_sf = nc.scalar.activation(sf[:, :nf], slab[:, 0:nf], AF.Sigmoid)
                    tt_ = work.tile([128, HS], F32, tag="tt_")
                    i_t = nc.scalar.activation(tt_[:, :nf], slab[:, HS:HS + nf],
                                               AF.Sigmoid, scale=negone_c[:])
                    sig_insts.append(i_sf)
                    sig_insts.append(i_t)
                    sp = work.tile([128, HS], BF16, tag="sp")
                    i_ln = nc.scalar.activation(sp[:, :nf], tt_[:, :nf], AF.Ln)
                    ln_insts.append(i_ln)
                    msg = work.tile([128, HS], BF16, tag="msg")
                    nc.vector.tensor_tensor(out=msg[:, :nf], in0=sf[:, :nf],
                                            in1=sp[:, :nf], op=ALU.mult)
                    if debug and l == 0 and b == 0 and s0 == 0:
                        dmsg = blkp.tile([128, 256], F32, tag="dmsg")
                        nc.vector.tensor_copy(dmsg[:], msg[:, 0:256])
                        nc.sync.dma_start(out=dbg_msg[:], in_=dmsg[:])
                    for jj in range(ns):
                        tg = tiles[s0 + jj][0]
                        U = work.tile([128, 128], BF16, tag="U")
                        nc.vector.tensor_scalar(
                            out=U[:], in0=iota_t[:], scalar1=dc_t[:, tg:tg + 1],
                            scalar2=None, op0=ALU.is_equal)
                        nc.tensor.matmul(
                            agg[:], lhsT=U[:], rhs=msg[:, jj * 128:(jj + 1) * 128],
                            start=(s0 + jj == 0), stop=(s0 + jj == nt - 1))
                # ---- block epilogue
                if debug and l == 0 and b == 0:
                    dagg = blkp.tile([128, D], F32, tag="dagg")
                    nc.vector.tensor_copy(dagg[:], agg[:])
                    nc.sync.dma_start(out=dbg_agg[:], in_=dagg[:])
                xres = blkp.tile([128, D], F32, tag="xres")
                if l == 0:
                    nc.sync.dma_start(out=xres[:], in_=x_in[b * 128:(b + 1) * 128, :])
                    xeff = xres
                else:
                    nc.sync.dma_start(out=xres[:], in_=y1_res[b * 128:(b + 1) * 128, :])
                    xe1 = blkp.tile([128, D], F32, tag="xe1")
                    nc.vector.tensor_tensor(out=xe1[:], in0=xres[:], in1=S1_t[:],
                                            op=ALU.mult)
                    xeff = blkp.tile([128, D], F32, tag="xe2")
                    nc.vector.tensor_tensor(out=xeff[:], in0=xe1[:], in1=B1_t[:],
                                            op=ALU.add)
                ysum = blkp.tile([128, D], F32, tag="ysum")
                nc.vector.tensor_tensor(out=ysum[:], in0=xeff[:], in1=agg[:],
                                        op=ALU.subtract)
                yy = blkp.tile([128, 2 * D], F32, tag="yy")
                y = yy[:, 0:D]
                nc.vector.tensor_scalar_mul(y, ysum[:], mask_t[:, b:b + 1])
                nc.vector.tensor_tensor(out=yy[:, D:2 * D], in0=y, in1=y,
                                        op=ALU.mult)
                nc.tensor.matmul(stats_ps[0:1, 0:256], lhsT=ones_c[:], rhs=yy[:],
                                 start=(b == 0), stop=(b == NBC - 1))
                if l == 0:
                    ybf = blkp.tile([128, D], BF16, tag="ybf")
                    nc.vector.tensor_copy(ybf[:], y)
                    nc.sync.dma_start(out=y_own[b * 128:(b + 1) * 128, :], in_=ybf[:])
                    nc.sync.dma_start(out=y1_res[b * 128:(b + 1) * 128, :], in_=y)
                    if debug:
                        nc.sync.dma_start(out=dbg_y1[b * 128:(b + 1) * 128, :], in_=y)
                else:
                    bm = blkp.tile([128, GW], F32, tag="bm")
                    nc.sync.dma_start(out=bm[:], in_=bmat_in[b * 128:(b + 1) * 128, :])
                    nc.tensor.matmul(pool_ps[:], lhsT=bm[:], rhs=y,
                                     start=(b == 0), stop=(b == NBC - 1))

        # ---- layer 1
        stats1 = ps_misc.tile([1, 256], F32, tag="stats")
        layer(0, xA, xB, x_own, stats1)

        # ---- BN1 stats AllReduce + fold
        st_sb = work.tile([1, 256], F32, tag="stsb")
        nc.vector.tensor_copy(st_sb[:], stats1[:])
        nc.sync.dma_start(out=ar_in[:], in_=st_sb[:])
        if debug:
            nc.sync.dma_start(out=dbg_st[:], in_=st_sb[:])
        nc.gpsimd.collective_compute(
            "AllReduce", ALU.add, replica_groups=rg,
            ins=[ar_in[:, :]], outs=[ar_out[:, :]])
        arv = work.tile([1, 256], F32, tag="arv")
        nc.sync.dma_start(out=arv[:], in_=ar_out[:])
        if debug:
            nc.sync.dma_start(out=dbg_ar[:], in_=arv[:])
        mean = work.tile([1, 128], F32, tag="mean")
        nc.vector.tensor_scalar_mul(mean[:], arv[0:1, 0:128], 1.0 / N)
        msq = work.tile([1, 128], F32, tag="msq")
        nc.vector.tensor_scalar_mul(msq[:], arv[0:1, 128:256], 1.0 / N)
        m2 = work.tile([1, 128], F32, tag="m2")
        nc.vector.tensor_tensor(out=m2[:], in0=mean[:], in1=mean[:], op=ALU.mult)
        var = work.tile([1, 128], F32, tag="var")
        nc.vector.tensor_tensor(out=var[:], in0=msq[:], in1=m2[:], op=ALU.subtract)
        lnv = work.tile([1, 128], F32, tag="lnv")
        nc.scalar.activation(lnv[:], var[:], AF.Ln, bias=eps_r[:])
        sraw = work.tile([1, 128], F32, tag="sraw")
        nc.scalar.activation(sraw[:], lnv[:], AF.Exp, scale=neghalf_r[:])
        s1r = work.tile([1, 128], F32, tag="s1r")
        nc.vector.tensor_tensor(out=s1r[:], in0=sraw[:], in1=bng_t[:], op=ALU.mult)
        ms1 = work.tile([1, 128], F32, tag="ms1")
        nc.vector.tensor_tensor(out=ms1[:], in0=mean[:], in1=s1r[:], op=ALU.mult)
        b1r = work.tile([1, 128], F32, tag="b1r")
        nc.vector.tensor_tensor(out=b1r[:], in0=bnb_t[:], in1=ms1[:], op=ALU.subtract)
        # row -> column via DRAM bounce
        nc.sync.dma_start(out=rowscr[0:1, :], in_=s1r[:])
        nc.sync.dma_start(out=rowscr[1:2, :], in_=b1r[:])
        s1c = work.tile([128, 1], F32, tag="s1c")
        nc.sync.dma_start(out=s1c[:], in_=rowscr[0:1, :].rearrange("o (p q) -> (o p) q", q=1))
        b1c = work.tile([128, 1], F32, tag="b1c")
        nc.sync.dma_start(out=b1c[:], in_=rowscr[1:2, :].rearrange("o (p q) -> (o p) q", q=1))
        b1cb = work.tile([128, 1], BF16, tag="b1cb")
        nc.vector.tensor_copy(b1cb[:], b1c[:])
        # S1/B1 broadcast matrices via K=1 outer product
        s1ps = ps_agg.tile([128, 128], F32, tag="agg")
        nc.tensor.matmul(s1ps[:], lhsT=ones_r[:], rhs=s1r[:], start=True, stop=True)
        nc.vector.tensor_copy(S1_t[:], s1ps[:])
        b1ps = ps_agg.tile([128, 128], F32, tag="agg")
        nc.tensor.matmul(b1ps[:], lhsT=ones_r[:], rhs=b1r[:], start=True, stop=True)
        nc.vector.tensor_copy(B1_t[:], b1ps[:])
        # fold BN1 into layer-2 weights: bias row first (raw W), then scale
        wd2, ws2, we2 = W[1]
        fold = ps_misc.tile([1, 256], F32, tag="stats")
        nc.tensor.matmul(fold[:], lhsT=b1cb[:], rhs=wd2[:], start=True, stop=False)
        nc.tensor.matmul(fold[:], lhsT=b1cb[:], rhs=ws2[:], start=False, stop=True)
        nc.vector.tensor_tensor(out=we2[64:65, :], in0=we2[64:65, :], in1=fold[:],
                                op=ALU.add)
        nc.vector.tensor_scalar_mul(wd2[:], wd2[:], s1c[:])
        nc.vector.tensor_scalar_mul(ws2[:], ws2[:], s1c[:])
        # AllGather y1 and split
        nc.gpsimd.collective_compute(
            "AllGather", ALU.bypass, replica_groups=rg,
            ins=[y_own[:, :]], outs=[ag_out[:, :]])
        nc.sync.dma_start(out=yA[:, :], in_=ag_out[0:HALF, :])
        nc.sync.dma_start(out=yB[:, :], in_=ag_out[HALF:NP, :])

        # ---- layer 2
        stats2 = ps_misc.tile([1, 256], F32, tag="stats")
        pool_ps = ps_pool.tile([GW, 128], F32, tag="pool")
        layer(1, yA, yB, y_own, stats2)

        # ---- epilogue
        st2 = work.tile([1, 256], F32, tag="st2")
        nc.vector.tensor_copy(st2[:], stats2[:])
        nc.sync.dma_start(out=stats2_out[:], in_=st2[:])
        pl = work.tile([GW, 128], F32, tag="pl")
        nc.vector.tensor_copy(pl[:], pool_ps[:])
        nc.sync.dma_start(out=pool_out[:], in_=pl[:])

    nc.compile()
    return nc


# ---------------------------------------------------------------------------
# entry points
# ---------------------------------------------------------------------------

def _run(inputs, G, trace=False, debug=False):
    x = np.asarray(inputs["x"], np.float32)
    prep = _prep(x, inputs["edge_index"], inputs["edge_attr"], inputs["batch"], G)
    nc = _build(prep, debug=debug)

    wcat = [_wcat(inputs["Wf0"], inputs["bf0"], inputs["Ws0"], inputs["bs0"]),
            _wcat(inputs["Wf1"], inputs["bf1"], inputs["Ws1"], inputs["bs1"])]
    iota = np.tile(np.arange(128, dtype=np.float32), (128, 1)).astype(BF16NP)
    in_maps = []
    for c in range(NCORES):
        im = {
            "x_shard": prep["x_shard"][c],
            "idx_src": prep["idx_src_w"][c],
            "idx_seq": prep["idx_seq_w"],
            "dstcol": prep["dstcol"][c],
            "ea_flat": prep["ea_flat"][c],
            "mask": prep["mask"][c],
            "Bmat": prep["Bmat"][c],
            "bn_g": np.asarray(inputs["g0"], np.float32).reshape(1, D),
            "bn_b": np.asarray(inputs["be0"], np.float32).reshape(1, D),
            "iota": iota,
            "iotaT": np.tile(np.arange(128, dtype=np.float32)[:, None],
                             (1, 128)).astype(BF16NP),
        }
        for l in range(2):
            im[f"wd{l}"], im[f"ws{l}"], im[f"we{l}"] = wcat[l]
        in_maps.append(im)

    res = run_bass_kernel_spmd(nc, in_maps, list(range(NCORES)), trace=trace)

    # host combine
    N = prep["N"]
    stats = sum(r["stats2"][0] for r in res.results)
    mean2 = stats[:D] / N
    var2 = stats[D:] / N - mean2 ** 2
    s2 = np.asarray(inputs["g1"], np.float32) / np.sqrt(var2 + EPS)
    b2 = np.asarray(inputs["be1"], np.float32) - mean2 * s2
    GW, glo, cnts = prep["GW"], prep["glo"], prep["cnts"]
    pool = np.zeros((G, D), np.float32)
    for c in range(NCORES):
        g0 = int(glo[c])
        hi = min(G, g0 + GW)
        pool[g0:hi] += res.results[c]["pool_out"][:hi - g0]
    out = np.zeros((G, D), np.float32)
    nz = cnts > 0
    out[nz] = s2[None, :] * pool[nz] / cnts[nz, None] + b2[None, :]
    return out, res


def kernel(**inputs):
    out, _ = _run(inputs, G=256)
    return out
